# revision 13
# baseline (speedup 1.0000x reference)
"""Causal depthwise conv1d (K=4) Trainium2 Bass kernel.

Problem: x [4, 4096, 2048] f32, W [2048, 4] f32.
  y[b,t,c] = sum_k W[c,k] * xpad[b, t+k, c]   (xpad = x left-padded 3 zeros in t)
  cache[b,c,k'] = x[b, 4093+k', c]

Sharding: 8 cores = batch(4) x time-halves(2). Core i: b=i//2, h=i%2.
Per-core shard: padded rows [2048h, 2048h+2051) of xpad_b -> y rows [2048h, 2048h+2048).

Per-core kernel (natural layout: t on partitions, c on free dim):
  - tile over t in steps of 125 outputs (128 input rows, 3-row halo)
  - A_k = x_tile * broadcast(W[:,k])      4 elementwise mults (DVE x3, GPSIMD x1)
  - psum_g += S_k.T @ A_k[:, 512g:512(g+1)]  banded 0/1 shift matmuls, PSUM-accumulated
  - ACT evicts psum -> y tile -> DMA out
Weight broadcast tiles built on-chip with a K=1 ones-matmul.
"""

import os
import sys

os.environ.setdefault("MYCRO_LOCAL_CACHE", "1")
sys.path.insert(0, "/opt/trn_rl_repo")

import numpy as np

B, T, C, K = 4, 4096, 2048, 4
NCORES = 8
TSH = T // 2          # 2048 output rows per core
PAD = K - 1           # 3
XROWS = TSH + PAD     # 2051 input rows per core
JC = 125              # output rows per tile
NG = 4                # 512-col matmul groups (C / 512)
CB = C // 128         # 16 c-blocks for cache transpose

_BUILT = None

# Elementwise-mult engine per tap: 'v' = vector(DVE), 'g' = gpsimd.
TAP_ENGINES = "vvvg"
MULT_DTYPE = "float32"   # 'float32' (exact) or 'bfloat16' (faster DVE)
CSPLIT = 1280            # tap-2 column split: [:CSPLIT] DVE, [CSPLIT:] GPSIMD


def _build_module():
    import concourse.mybir as mybir
    from concourse import bacc
    from concourse.tile import TileContext

    f32 = mybir.dt.float32
    bf16 = mybir.dt.bfloat16
    mdt = f32 if MULT_DTYPE == "float32" else bf16

    nc = bacc.Bacc(None, target_bir_lowering=False, debug=False)

    x_d = nc.dram_tensor("x", [XROWS, C], f32, kind="ExternalInput")
    wt_d = nc.dram_tensor("wt", [1, K * C], f32, kind="ExternalInput")
    s_d = nc.dram_tensor("s", [128, K, 128], f32, kind="ExternalInput")
    y_d = nc.dram_tensor("y", [TSH, C], f32, kind="ExternalOutput")
    cache_d = nc.dram_tensor("cache", [C, PAD], f32, kind="ExternalOutput")

    with TileContext(nc) as tc:
        with (
            tc.tile_pool(name="const", bufs=1) as const,
            tc.tile_pool(name="xpool", bufs=3) as xpool,
            tc.tile_pool(name="apool", bufs=8) as apool,
            tc.tile_pool(name="ypool", bufs=3) as ypool,
            tc.tile_pool(name="pspool", bufs=8, space="PSUM") as pspool,
        ):
            # --- constants ---
            s_t = const.tile([128, K, 128], mdt)
            dma_s = nc.gpsimd if mdt != f32 else nc.sync
            dma_s.dma_start(s_t[:, :, :], s_d[:, :, :])

            wtt = const.tile([1, K * C], f32)
            nc.sync.dma_start(wtt[:1, :], wt_d[:1, :])

            ones = const.tile([1, 128], f32)
            nc.vector.memset(ones[:1, :], 1.0)

            # broadcast W[:,k] across 128 partitions: wb[p, k, c] = W[c, k]
            wb = const.tile([128, K, C], mdt)
            for k in range(K):
                for g in range(NG):
                    ps = pspool.tile([128, 512], f32, tag="ps", bufs=2)
                    nc.tensor.matmul(
                        ps[:128, :512],
                        ones[:1, :128],
                        wtt[:1, k * C + g * 512 : k * C + (g + 1) * 512],
                        start=True,
                        stop=True,
                    )
                    nc.scalar.copy(wb[:, k, g * 512 : (g + 1) * 512], ps[:128, :512])

            # --- main tiling loop over output rows ---
            for a in range(0, TSH, JC):
                jc = min(JC, TSH - a)
                qc = jc + PAD
                xin = xpool.tile([128, C], f32, tag="xin")
                nc.sync.dma_start(xin[:qc, :], x_d[a : a + qc, :])

                if mdt != f32:
                    xsrc = xpool.tile([128, C], mdt, tag="xbf")
                    nc.scalar.copy(xsrc[:qc, :], xin[:qc, :])
                else:
                    xsrc = xin

                # tap->engine split balancing DVE (faster) vs GPSIMD:
                # k=0,1 DVE full; k=2 split by columns; k=3 GPSIMD full.
                aks = []
                for k in range(K):
                    ak = apool.tile([128, C], mdt, tag="ak")
                    if k == 2:
                        nc.vector.tensor_mul(
                            ak[:qc, :CSPLIT], xsrc[:qc, :CSPLIT], wb[:qc, k, :CSPLIT]
                        )
                        nc.gpsimd.tensor_mul(
                            ak[:qc, CSPLIT:], xsrc[:qc, CSPLIT:], wb[:qc, k, CSPLIT:]
                        )
                    else:
                        eng = nc.vector if TAP_ENGINES[k] == "v" else nc.gpsimd
                        eng.tensor_mul(ak[:qc, :], xsrc[:qc, :], wb[:qc, k, :])
                    aks.append(ak)

                ps = pspool.tile([128, C], f32, tag="ps", bufs=2)
                for k in range(K):
                    for g in range(NG):
                        nc.tensor.matmul(
                            ps[:jc, g * 512 : (g + 1) * 512],
                            s_t[:qc, k, :jc],
                            aks[k][:qc, g * 512 : (g + 1) * 512],
                            start=(k == 0),
                            stop=(k == K - 1),
                        )

                yout = ypool.tile([128, C], f32, tag="yout")
                nc.scalar.copy(yout[:jc, :], ps[:jc, :])
                nc.scalar.dma_start(y_d[a : a + jc, :], yout[:jc, :])

            # --- cache output: cache[c, k'] = x_sh[2048+k', c] ---
            xc = xpool.tile([128, C], f32, tag="xin")
            nc.sync.dma_start(xc[:PAD, :], x_d[TSH : TSH + PAD, :])
            if mdt != f32:
                xcm = xpool.tile([128, C], mdt, tag="xbf")
                nc.scalar.copy(xcm[:PAD, :], xc[:PAD, :])
            else:
                xcm = xc
            psc = pspool.tile([128, CB, PAD], f32, tag="ps", bufs=2)
            for cb in range(CB):
                nc.tensor.matmul(
                    psc[:128, cb, :PAD],
                    xcm[:PAD, cb * 128 : (cb + 1) * 128],
                    s_t[:PAD, 0, :PAD],
                    start=True,
                    stop=True,
                )
            ctile = ypool.tile([128, CB, PAD], f32, tag="ct")
            nc.scalar.copy(ctile[:, :, :], psc[:, :, :])
            cache_view = cache_d[:, :].rearrange("(cb p) k -> p cb k", p=128)
            nc.sync.dma_start(cache_view, ctile[:, :, :])

    nc.compile()
    return nc


def _get_module():
    global _BUILT
    if _BUILT is None:
        _BUILT = _build_module()
    return _BUILT


def _make_inputs(x, W):
    x = np.asarray(x, dtype=np.float32)
    W = np.asarray(W, dtype=np.float32)
    wt = np.ascontiguousarray(W.T.reshape(1, K * C))
    S = np.zeros((128, K, 128), dtype=np.float32)
    for k in range(K):
        for j in range(128 - k):
            S[j + k, k, j] = 1.0
    in_maps = []
    for i in range(NCORES):
        b, h = i // 2, i % 2
        xpad = np.concatenate(
            [np.zeros((PAD, C), dtype=np.float32), x[b]], axis=0
        )  # [4099, C]
        xs = np.ascontiguousarray(xpad[TSH * h : TSH * h + XROWS])
        in_maps.append({"x": xs, "wt": wt, "s": S})
    return in_maps


def _gather(results):
    y = np.empty((B, T, C), dtype=np.float32)
    cache = np.empty((B, C, PAD), dtype=np.float32)
    for i in range(NCORES):
        b, h = i // 2, i % 2
        y[b, TSH * h : TSH * (h + 1), :] = results[i]["y"]
        if h == 1:
            cache[b] = results[i]["cache"]
    return y, cache


def run(x, W, trace=False, trace_cores=None):
    from concourse.bass_utils import run_bass_kernel_spmd

    nc = _get_module()
    in_maps = _make_inputs(x, W)
    kw = {}
    if trace:
        kw = dict(trace=True, trace_cores=trace_cores or list(range(NCORES)))
    try:
        res = run_bass_kernel_spmd(nc, in_maps, core_ids=list(range(NCORES)), **kw)
    except ModuleNotFoundError:
        # NTFF profile hook unavailable under this axon build - run untraced.
        res = run_bass_kernel_spmd(nc, in_maps, core_ids=list(range(NCORES)))
    y, cache = _gather(res.results)
    return (y, cache), res


def kernel(x, W):
    (y, cache), _ = run(x, W, trace=False)
    return y, cache
